# revision 27
# baseline (speedup 1.0000x reference)
"""Trainium2 Bass kernel: causal multi-head attention block (B=2, S=2048, D=2048, H=16).

Sharding: tensor-parallel over heads. Each of the 8 cores owns 2 heads:
  - wq/wk/wv column-sharded (256 output features = 2 heads per core)
  - attention computed locally per head
  - AllGather of attention output (one per 512-token chunk), wo
    row-sharded so each core produces a 256-wide column slice of the
    final output.

Structure: one fused pipeline over 8 token chunks of 512. Q/K are
computed FEATURE-MAJOR (wcat stationary, xT streaming) so RoPE happens
in [hd, token] layout with no transposes: the head dim is permuted
host-side to [reals | imags] blocks, RoPE = two DVE muls off PSUM
(cos/sin tables with signs baked in) + a tiny 64-partition-rotation
matmul + one DVE add writing qT/kT directly. V stays token-major
(x stationary). Scores computed transposed (sT[j,i]); exp on scalar
engine; softmax denominators via DVE accumulation of exp tiles + one
ones-matmul per (head, chunk); head1's scores interleave with head0's
PV matmuls so the PE rides out the exp drains. The projection (psum in
the otherwise-idle V bank) is delayed four chunks so the skew-delayed
early AllGathers never stall the PE, and its matmuls fill PE slots
between exp-gated tiles; the tail runs the last four projections
back-to-back, hiding the final AllGather. The first three chunks issue
QKV dt-outer (borrowing score banks) so compute starts while x/weight
tiles are still streaming in.
"""

import sys

sys.path.insert(0, "/opt/trn_rl_repo")

import numpy as np
import ml_dtypes

B, S, D, H = 2, 2048, 2048, 16
HD = 128          # head dim
NCORES = 8
HPC = H // NCORES  # heads per core = 2
ESH = HPC * HD     # per-core feature shard = 256
T = B * S          # flattened tokens = 4096
NT = T // 128      # token tiles = 32
ND = D // 128      # feature tiles = 16
NCHUNK = T // 512  # pipeline chunks = 8
SCALE = 1.0 / np.sqrt(HD)

_cache = {}


def _build_nc():
    import concourse.bass as bass
    import concourse.mybir as mybir
    import concourse.tile as tile
    from concourse import bacc

    f32 = mybir.dt.float32
    bf16 = mybir.dt.bfloat16

    nc = bacc.Bacc(None, target_bir_lowering=False, num_devices=NCORES)

    # ---- kernel I/O (per-core shards, prepared on host) ----
    xTd = nc.declare_dram_parameter("xT", [NCHUNK * 128, ND * 512], bf16,
                                    isOutput=False)
    wcatT = nc.declare_dram_parameter("wcatT", [128, ND * 3 * ESH], bf16,
                                      isOutput=False)
    woT = nc.declare_dram_parameter("woT", [D, ESH], bf16, isOutput=False)
    cosf = nc.declare_dram_parameter("cosf", [HD, S], bf16, isOutput=False)
    sinf = nc.declare_dram_parameter("sinf", [HD, S], bf16, isOutput=False)
    rotmd = nc.declare_dram_parameter("rotm", [128, 128], bf16, isOutput=False)
    maskdT = nc.declare_dram_parameter("maskdT", [128, 128], bf16, isOutput=False)
    yT = nc.declare_dram_parameter("out", [ESH, T], f32, isOutput=True)

    # collective bounce buffers (internal DRAM), one pair per chunk
    agin = [nc.dram_tensor(f"agin{i}", [ESH, 512], bf16)
            for i in range(NCHUNK)]
    agout = [nc.dram_tensor(f"agout{i}", [D, 512], bf16,
                            addr_space="Shared")
             for i in range(NCHUNK)]
    # tiny warmup collective: absorbs cross-core startup skew behind the
    # first chunk's compute instead of stalling the first real AllGather
    agin_w = nc.dram_tensor("aginw", [128, 8], bf16)
    agout_w = nc.dram_tensor("agoutw", [128 * NCORES, 8], bf16,
                             addr_space="Shared")

    with tile.TileContext(nc) as tc:
        with (
            tc.tile_pool(name="const", bufs=1) as constp,
            tc.tile_pool(name="wpool", bufs=1) as wpool,
            tc.tile_pool(name="qkv", bufs=1) as qkvp,
            tc.tile_pool(name="xt", bufs=2) as xtp,
            tc.tile_pool(name="rope", bufs=8) as ropep,
            tc.tile_pool(name="pt", bufs=18) as ptp,
            tc.tile_pool(name="acc", bufs=2) as accp,
            tc.tile_pool(name="accb", bufs=2) as accbp,
            tc.tile_pool(name="sm", bufs=2) as smp,
            tc.tile_pool(name="ot", bufs=3) as otp,
            tc.tile_pool(name="ys", bufs=2) as ysp,
            tc.tile_pool(name="yrhs", bufs=30) as yrhsp,
            tc.tile_pool(name="psQ", bufs=2, space="PSUM") as psQ,
            tc.tile_pool(name="psQb", bufs=1, space="PSUM") as psQb,
            tc.tile_pool(name="psS", bufs=3, space="PSUM") as psS,
            tc.tile_pool(name="psB", bufs=2, space="PSUM") as psB,
        ):
            # ---- warmup collective first: pure rendezvous ----
            nc.gpsimd.collective_compute(
                "AllGather",
                mybir.AluOpType.bypass,
                replica_groups=[list(range(NCORES))],
                ins=[agin_w.ap().opt()],
                outs=[agout_w.ap().opt()],
            )

            # rope tables, feature-major: [128 hd-part, S]
            cos_sb = constp.tile([128, S], bf16, tag="cos")
            sin_sb = constp.tile([128, S], bf16, tag="sin")
            nc.sync.dma_start(cos_sb[:], cosf[:, :])
            nc.sync.dma_start(sin_sb[:], sinf[:, :])

            ones = constp.tile([128, 128], bf16, tag="ones")
            nc.vector.memset(ones[:], 1.0)
            # 64-partition rotation matrix (RoPE pair swap in [r|i] layout)
            rotm = constp.tile([128, 128], bf16, tag="rotm")
            nc.sync.dma_start(rotm[:], rotmd[:, :])

            # resident weights: wcatT tiles [128d, 768], paired with the
            # first chunk's xT tiles so chunk-0 compute starts immediately
            xt_tiles = {}
            xt0 = xtp.tile([128, ND * 512], bf16, tag="xt", name="xt0")
            xt_tiles[0] = xt0
            # wcat packed [128, ND*768]: partition p holds all 16 d-tiles'
            # rows; loaded in 4-dt groups (6KB lines) interleaved with the
            # first chunk's x groups (8KB lines) so chunk-0 compute starts
            # while both stream in
            wcat_t = wpool.tile([128, ND * 3 * ESH], bf16, tag="wc",
                                name="wcat_t")
            wcat_sb = [wcat_t[:, dt * 768:(dt + 1) * 768] for dt in range(ND)]
            for g in range(4):
                nc.sync.dma_start(
                    wcat_t[:, g * 3072:(g + 1) * 3072],
                    wcatT[0:128, g * 3072:(g + 1) * 3072])
                nc.scalar.dma_start(
                    xt0[:, g * 2048:(g + 1) * 2048],
                    xTd[0:128, g * 2048:(g + 1) * 2048])

            # chunks 0-2 fill in 4-dt groups (compute starts while tiles
            # stream in); later chunks load as one dma (16KB lines)
            def fetch_chunk(tc_i):
                xt = xtp.tile([128, ND * 512], bf16, tag="xt", name="xtc")
                xt_tiles[tc_i] = xt
                r0 = tc_i * 128
                if tc_i < 3:
                    for g in range(4):
                        eng = nc.sync if g % 2 == 0 else nc.scalar
                        eng.dma_start(
                            xt[:, g * 2048:(g + 1) * 2048],
                            xTd[r0:r0 + 128, g * 2048:(g + 1) * 2048])
                else:
                    nc.scalar.dma_start(xt[:], xTd[r0:r0 + 128, :])

            fetch_chunk(1)

            # transposed diagonal mask block [j-part, i-free]; every
            # 128x128 diagonal block of the causal mask is identical
            maskd_sb = constp.tile([128, 128], bf16, tag="maskd")
            nc.scalar.dma_start(maskd_sb[:], maskdT[:, :])

            # wo tiles: loaded lazily (first needed by proj at chunk 4)
            wo_sb = []

            def load_wo():
                for dt in range(ND):
                    t_ = wpool.tile([128, ESH], bf16, tag=f"wo{dt}",
                                    name=f"wo{dt}")
                    nc.scalar.dma_start(t_[:], woT[dt * 128:(dt + 1) * 128, :])
                    wo_sb.append(t_)

            # persistent attention operands (bf16)
            # qT/kT: [(h,b)] -> [128hd, S]  (head-major, feature on partitions)
            # v    : [(h,b)] -> [128s-in-tile, S/128 * HD] natural blocks
            qT = {}
            kT = {}
            vN = {}
            for h in range(HPC):
                for b in range(B):
                    qT[h, b] = qkvp.tile([128, S], bf16, tag=f"q{h}{b}",
                                         name=f"qT{h}{b}")
                    kT[h, b] = qkvp.tile([128, S], bf16, tag=f"k{h}{b}",
                                         name=f"kT{h}{b}")
                    vN[h, b] = qkvp.tile([128, S // 128 * HD], bf16,
                                         tag=f"v{h}{b}", name=f"vN{h}{b}")

            # ---------------- QKV + RoPE ----------------
            # feature-major rope: psum q tile [128 hd, 512 tok] for one head;
            # hd permuted host-side to [64 reals | 64 imags].
            #   qc = psum * cos ; qs = psum * sin2  (sin2 = [+sin | -sin])
            #   dst = qc + rotate64(qs)
            # the 64-partition rotation runs on the PE (tiny matmul with a
            # permutation stationary) since DVE ops can't cross partitions;
            # the rot matmul is emitted a group later so its DVE deps are
            # ready by the time the PE reaches it.
            def rope_muls(ps_ap, span):
                qc = ropep.tile([128, 512], bf16, tag="qc")
                qs = ropep.tile([128, 512], bf16, tag="qs")
                nc.vector.tensor_mul(qc[:], ps_ap, cos_sb[:, span])
                nc.vector.tensor_mul(qs[:], ps_ap, sin_sb[:, span])
                return qc, qs

            def rope_rot(qc, qs, dst, span):
                prot = psQ.tile([128, 512], f32, tag="pa", name="prot")
                nc.tensor.matmul(prot[:], rotm[:], qs[:],
                                 start=True, stop=True)
                nc.vector.tensor_add(dst[:, span], qc[:], prot[:])

            def drain_v_pair(pv_ap, b, st0):
                # pv_ap holds [128 tok, 2tt * 2h * HD/... ] = [:,0:256]=tt0
                for half, st in ((0, st0), (1, st0 + 1)):
                    for h in range(HPC):
                        nc.scalar.activation(
                            vN[h, b][:, st * HD:(st + 1) * HD],
                            pv_ap[:, half * 256 + h * HD:
                                  half * 256 + (h + 1) * HD],
                            mybir.ActivationFunctionType.Copy)

            # dt-outer QKV: all six accumulation groups advance together
            # per dt (borrows psS banks) so compute starts as tiles arrive.
            # NOTE: start=True clears has_written for the WHOLE psum bank;
            # when two sub-groups share a bank only the first may set start.
            def qkv_chunk_dtouter(tc_i):
                b = tc_i // (S // 512)
                st0 = (tc_i * 4) % (S // 128)
                span = slice((tc_i % 4) * 512, (tc_i % 4) * 512 + 512)
                pq = [psQ.tile([128, 512], f32, tag="pa", name=f"pq{i}")
                      for i in range(2)]
                pk = [psS.tile([128, 512], f32, tag="S", name=f"pk{i}")
                      for i in range(2)]
                pva = psQb.tile([128, 512], f32, tag="pb")
                pvb = psS.tile([128, 512], f32, tag="S")
                for dt in range(ND):
                    xt = xt_tiles[tc_i][:, dt * 512:(dt + 1) * 512]
                    st_, sp_ = (dt == 0), (dt == ND - 1)
                    for f in range(2):
                        nc.tensor.matmul(
                            pq[f][:], wcat_sb[dt][:, f * 128:(f + 1) * 128],
                            xt[:], start=st_, stop=sp_)
                    for f in range(2):
                        nc.tensor.matmul(
                            pk[f][:],
                            wcat_sb[dt][:, 256 + f * 128:256 + (f + 1) * 128],
                            xt[:], start=st_, stop=sp_)
                    for tt in range(4):
                        dst = pva if tt < 2 else pvb
                        nc.tensor.matmul(
                            dst[:, (tt % 2) * 256:(tt % 2) * 256 + 256],
                            xt[:, tt * 128:(tt + 1) * 128],
                            wcat_sb[dt][:, 512:768],
                            start=(st_ and tt % 2 == 0), stop=sp_,
                            skip_group_check=True)
                mres = []
                for h in range(HPC):
                    mres.append((rope_muls(pq[h][:], span), qT[h, b]))
                for h in range(HPC):
                    mres.append((rope_muls(pk[h][:], span), kT[h, b]))
                for (qc, qs), dst in mres:
                    rope_rot(qc, qs, dst, span)
                drain_v_pair(pva[:], b, st0)
                drain_v_pair(pvb[:], b, st0 + 2)

            # ---------------- attention phases ----------------
            # one scores j-tile: matmul + (diag) mask add + exp + denominator
            # accumulation on DVE
            def score_tile(h, b, c, jt, acc, pts):
                lo = max(0, jt * 128 - c * 512)
                ps = psS.tile([128, 512], f32, tag="S")
                nc.tensor.matmul(
                    ps[:, lo:512],
                    kT[h, b][:, jt * 128:(jt + 1) * 128],
                    qT[h, b][:, c * 512 + lo:(c + 1) * 512],
                    start=True, stop=True)
                if jt >= 4 * c:
                    # diagonal block: add transposed mask
                    dl = jt * 128 - c * 512
                    nc.vector.tensor_add(
                        ps[:, dl:dl + 128], ps[:, dl:dl + 128],
                        maskd_sb[:])
                pt = ptp.tile([128, 512], bf16, tag="pt")
                nc.scalar.activation(
                    pt[:, lo:512], ps[:, lo:512],
                    mybir.ActivationFunctionType.Exp, scale=SCALE)
                if jt == 0:
                    nc.vector.tensor_scalar_mul(
                        acc[:, lo:512], pt[:, lo:512], 1.0)
                else:
                    nc.vector.tensor_add(
                        acc[:, lo:512], acc[:, lo:512], pt[:, lo:512])
                pts.append((pt, lo))

            # PV accumulation matmuls for one head (interleavable thunks)
            def pv_mms(h, b, c, pts, po):
                J = 4 * (c + 1)
                out = []
                for jt in range(J):
                    pt, lo = pts[jt]
                    out.append(lambda jt=jt, pt=pt, lo=lo: nc.tensor.matmul(
                        po[:, lo:512], vN[h, b][:, jt * HD:(jt + 1) * HD],
                        pt[:, lo:512], start=(jt == 0), stop=(jt == J - 1),
                        skip_group_check=True))
                return out

            # denominator matmul (cross-partition sum of acc, replicated)
            def denom_mm(acc):
                accb = accbp.tile([128, 512], bf16, tag="accb")
                nc.vector.tensor_scalar_mul(accb[:], acc[:], 1.0)
                pr = psB.tile([128, 512], f32, tag="B")
                nc.tensor.matmul(pr[:], ones[:], accb[:],
                                 start=True, stop=True)
                return pr

            # reciprocal -> normalize -> bounce write (per head half)
            def finish_phase(ci, h, pr, po):
                rbc = smp.tile([128, 512], f32, tag="rbc")
                nc.vector.reciprocal_approx_fast(rbc[:], pr[:])
                ot = otp.tile([128, 512], bf16, tag="ot")
                nc.vector.tensor_mul(ot[:], po[:], rbc[:])
                nc.scalar.dma_start(agin[ci][h * 128:(h + 1) * 128, :], ot[:])

            def all_gather(ci):
                nc.gpsimd.collective_compute(
                    "AllGather",
                    mybir.AluOpType.bypass,
                    replica_groups=[list(range(NCORES))],
                    ins=[agin[ci].ap().opt()],
                    outs=[agout[ci].ap().opt()],
                )

            # ---------------- output projection ----------------
            # NOTE: these loads depend on an AllGather; they must go on the
            # sync queue ONLY. A dma_start waiting on a collective would
            # head-of-line-block every later instruction on its engine
            # queue -- on the scalar queue that stalls all exp drains and
            # with them the whole pipeline when the fabric is slow.
            def rhs_load(k):
                rhs_tiles = []
                for et in range(ND):
                    rt = yrhsp.tile([128, 512], bf16, tag="yr")
                    nc.sync.dma_start(
                        rt[:], agout[k][et * 128:(et + 1) * 128, :])
                    rhs_tiles.append(rt)
                return rhs_tiles

            # projection emitted as thunks so its matmuls can fill PE gaps
            # between exp-gated score/PV tiles; psum borrowed from the v
            # bank (psQb), which is idle outside the QKV phase
            def project_thunks(k, rhs_tiles):
                thunks = []
                for ft in range(ESH // 128):
                    box = {}
                    for et in range(ND):
                        def mm(ft=ft, et=et, box=box):
                            if et == 0:
                                box["py"] = psQb.tile([128, 512], f32,
                                                      tag="pb", name="py")
                            nc.tensor.matmul(
                                box["py"][:],
                                wo_sb[et][:, ft * 128:(ft + 1) * 128],
                                rhs_tiles[et][:],
                                start=(et == 0), stop=(et == ND - 1))
                        thunks.append(mm)

                    def drain(ft=ft, box=box, k=k):
                        ys = ysp.tile([128, 512], f32, tag="ys")
                        nc.scalar.activation(
                            ys[:], box["py"][:],
                            mybir.ActivationFunctionType.Copy)
                        nc.scalar.dma_start(
                            yT[ft * 128:(ft + 1) * 128,
                               k * 512:(k + 1) * 512],
                            ys[:])
                    thunks.append(drain)
                return thunks

            # ---------------- fused pipeline ----------------
            rhs_pre = {}
            for ci in range(NCHUNK):
                b, c = divmod(ci, S // 512)
                J = 4 * (c + 1)
                if 1 <= ci and ci + 1 < NCHUNK:
                    fetch_chunk(ci + 1)
                if ci == 3:
                    load_wo()
                if ci == 4:
                    rhs_pre[0] = rhs_load(0)
                filler = []
                for k in {5: [0], 6: [1, 2, 3]}.get(ci, []):
                    filler += project_thunks(
                        k, rhs_pre.pop(k) if k in rhs_pre else rhs_load(k))

                def pop_filler(n=1):
                    for _ in range(min(n, len(filler))):
                        filler.pop(0)()

                qkv_chunk_dtouter(ci)
                # head0 scores, with projection filler between tiles
                last = ci == NCHUNK - 1
                pts0 = []
                acc0 = accp.tile([128, 512], f32, tag="acc")
                for jt in range(J):
                    score_tile(0, b, c, jt, acc0, pts0)
                    if not last:
                        pop_filler()
                pr0 = denom_mm(acc0)
                # head1 scores interleaved with head0 PV
                po0 = psB.tile([128, 512], f32, tag="B")
                pv0 = pv_mms(0, b, c, pts0, po0)
                pts1 = []
                acc1 = accp.tile([128, 512], f32, tag="acc")
                for jt in range(J):
                    score_tile(1, b, c, jt, acc1, pts1)
                    pv0[jt]()
                finish_phase(ci, 0, pr0, po0)
                pr1 = denom_mm(acc1)
                # head1 PV interleaved with projection filler
                po1 = psB.tile([128, 512], f32, tag="B")
                pv1 = pv_mms(1, b, c, pts1, po1)
                for jt in range(J):
                    pv1[jt]()
                    if not last:
                        pop_filler()
                finish_phase(ci, 1, pr1, po1)
                all_gather(ci)
                pop_filler(len(filler))
                if last:
                    for k in (NCHUNK - 4, NCHUNK - 3):
                        rhs_pre[k] = rhs_load(k)
            for k in range(NCHUNK - 4, NCHUNK):
                rhs = rhs_pre.pop(k) if k in rhs_pre else rhs_load(k)
                for th in project_thunks(k, rhs):
                    th()
    nc.finalize()
    return nc


def _prep_inputs(x, wq, wk, wv, wo, freqs_cos, freqs_sin, mask):
    bf16 = ml_dtypes.bfloat16
    xf = np.ascontiguousarray(x.reshape(T, D).T).astype(bf16)
    # pre-tiled [NCHUNK, 128, ND, 512]: each SBUF partition's whole-chunk
    # data (16 dt-slices) is one contiguous 16KB DRAM row, so a chunk
    # loads as ONE dma with 16KB lines instead of 16 dmas of 1KB lines
    xf = np.ascontiguousarray(
        xf.reshape(ND, 128, NCHUNK, 512).transpose(2, 1, 0, 3)
    ).reshape(NCHUNK * 128, ND * 512)
    # rope tables, feature-major [128, S]:
    #   rows 0..63 = pair j (reals), rows 64..127 = pair j (imags)
    #   cos rows duplicate cos[:, j]; sin rows carry the rotation signs:
    #   sin2[0:64] = +sin (applied to swapped imag->real term read from
    #   the imag block), sin2[64:128] = -sin.
    cosf = np.concatenate([freqs_cos.T, freqs_cos.T], axis=0).astype(bf16)
    sinf = np.concatenate([freqs_sin.T, -freqs_sin.T], axis=0).astype(bf16)
    # 64-partition rotation: out[p] = in[(p+64)%128] as out = rotm.T @ in
    rotmat = np.zeros((128, 128), dtype=bf16)
    rotmat[(np.arange(128) + 64) % 128, np.arange(128)] = 1
    # one transposed diagonal mask block (all diagonal blocks identical)
    mdT = np.ascontiguousarray(mask[0:128, 0:128].T).astype(bf16)
    # head-dim permutation for q/k: [evens | odds] per head
    hdperm = np.concatenate([np.arange(0, HD, 2), np.arange(1, HD, 2)])
    in_maps = []
    for cidx in range(NCORES):
        sl = slice(cidx * ESH, (cidx + 1) * ESH)
        wqc, wkc = wq[sl, :], wk[sl, :]
        qperm = np.concatenate([h * HD + hdperm for h in range(HPC)])
        wcatT = np.concatenate(
            [wqc[qperm, :].T, wkc[qperm, :].T, wv[sl, :].T],
            axis=1).astype(bf16)
        # pack [D, 768] -> [128, ND*768]: partition p holds its 16 tile-rows
        wcatT = np.ascontiguousarray(
            wcatT.reshape(ND, 128, 768).transpose(1, 0, 2)
        ).reshape(128, ND * 768)
        woTc = np.ascontiguousarray(wo[sl, :].T).astype(bf16)
        in_maps.append({
            "xT": xf,
            "wcatT": np.ascontiguousarray(wcatT),
            "woT": woTc,
            "cosf": cosf,
            "sinf": sinf,
            "rotm": rotmat,
            "maskdT": mdT,
        })
    return in_maps


def kernel(x, wq, wk, wv, wo, freqs_cos, freqs_sin, mask, start_pos):
    from concourse.bass_utils import run_bass_kernel_spmd

    x = np.asarray(x, dtype=np.float32)
    in_maps = _prep_inputs(
        np.asarray(x, np.float32), np.asarray(wq, np.float32),
        np.asarray(wk, np.float32), np.asarray(wv, np.float32),
        np.asarray(wo, np.float32), np.asarray(freqs_cos, np.float32),
        np.asarray(freqs_sin, np.float32), np.asarray(mask, np.float32))

    if "nc" not in _cache:
        _cache["nc"] = _build_nc()
    res = run_bass_kernel_spmd(_cache["nc"], in_maps, core_ids=list(range(NCORES)))
    _cache["last_result"] = res

    y = np.empty((T, D), dtype=np.float32)
    for c in range(NCORES):
        y[:, c * ESH:(c + 1) * ESH] = np.asarray(res.results[c]["out"]).T
    return y.reshape(B, S, D)


# revision 28
# speedup vs baseline: 1.0224x; 1.0224x over previous
"""Trainium2 Bass kernel: causal multi-head attention block (B=2, S=2048, D=2048, H=16).

Sharding: tensor-parallel over heads. Each of the 8 cores owns 2 heads:
  - wq/wk/wv column-sharded (256 output features = 2 heads per core)
  - attention computed locally per head
  - AllGather of attention output (one per 512-token chunk), wo
    row-sharded so each core produces a 256-wide column slice of the
    final output.

Structure: one fused pipeline over 8 token chunks of 512. Q/K are
computed FEATURE-MAJOR (wcat stationary, xT streaming) so RoPE happens
in [hd, token] layout with no transposes: the head dim is permuted
host-side to [reals | imags] blocks, RoPE = two DVE muls off PSUM
(cos/sin tables with signs baked in) + a tiny 64-partition-rotation
matmul + one DVE add writing qT/kT directly. V stays token-major
(x stationary). Scores computed transposed (sT[j,i]); exp on scalar
engine; softmax denominators via DVE accumulation of exp tiles + one
ones-matmul per (head, chunk); head1's scores interleave with head0's
PV matmuls so the PE rides out the exp drains. The projection (psum in
the otherwise-idle V bank) is delayed four chunks so the skew-delayed
early AllGathers never stall the PE, and its matmuls fill PE slots
between exp-gated tiles; the tail runs the last four projections
back-to-back, hiding the final AllGather. The first three chunks issue
QKV dt-outer (borrowing score banks) so compute starts while x/weight
tiles are still streaming in.
"""

import sys

sys.path.insert(0, "/opt/trn_rl_repo")

import numpy as np
import ml_dtypes

B, S, D, H = 2, 2048, 2048, 16
HD = 128          # head dim
NCORES = 8
HPC = H // NCORES  # heads per core = 2
ESH = HPC * HD     # per-core feature shard = 256
T = B * S          # flattened tokens = 4096
NT = T // 128      # token tiles = 32
ND = D // 128      # feature tiles = 16
NCHUNK = T // 512  # pipeline chunks = 8
SCALE = 1.0 / np.sqrt(HD)

_cache = {}


def _build_nc():
    import concourse.bass as bass
    import concourse.mybir as mybir
    import concourse.tile as tile
    from concourse import bacc

    f32 = mybir.dt.float32
    bf16 = mybir.dt.bfloat16

    nc = bacc.Bacc(None, target_bir_lowering=False, num_devices=NCORES)

    # ---- kernel I/O (per-core shards, prepared on host) ----
    xTd = nc.declare_dram_parameter("xT", [NCHUNK * 128, ND * 512], bf16,
                                    isOutput=False)
    wcatT = nc.declare_dram_parameter("wcatT", [128, ND * 3 * ESH], bf16,
                                      isOutput=False)
    woT = nc.declare_dram_parameter("woT", [D, ESH], bf16, isOutput=False)
    cosf = nc.declare_dram_parameter("cosf", [HD, S], bf16, isOutput=False)
    sinf = nc.declare_dram_parameter("sinf", [HD, S], bf16, isOutput=False)
    rotmd = nc.declare_dram_parameter("rotm", [128, 128], bf16, isOutput=False)
    maskdT = nc.declare_dram_parameter("maskdT", [128, 128], bf16, isOutput=False)
    yT = nc.declare_dram_parameter("out", [ESH, T], f32, isOutput=True)

    # collective bounce buffers (internal DRAM), one pair per chunk
    agin = [nc.dram_tensor(f"agin{i}", [ESH, 512], bf16)
            for i in range(NCHUNK)]
    agout = [nc.dram_tensor(f"agout{i}", [D, 512], bf16,
                            addr_space="Shared")
             for i in range(NCHUNK)]
    # tiny warmup collective: absorbs cross-core startup skew behind the
    # first chunk's compute instead of stalling the first real AllGather
    agin_w = nc.dram_tensor("aginw", [128, 8], bf16)
    agout_w = nc.dram_tensor("agoutw", [128 * NCORES, 8], bf16,
                             addr_space="Shared")

    with tile.TileContext(nc) as tc:
        with (
            tc.tile_pool(name="const", bufs=1) as constp,
            tc.tile_pool(name="wpool", bufs=1) as wpool,
            tc.tile_pool(name="qkv", bufs=1) as qkvp,
            tc.tile_pool(name="xt", bufs=2) as xtp,
            tc.tile_pool(name="rope", bufs=8) as ropep,
            tc.tile_pool(name="pt", bufs=18) as ptp,
            tc.tile_pool(name="acc", bufs=2) as accp,
            tc.tile_pool(name="accb", bufs=2) as accbp,
            tc.tile_pool(name="sm", bufs=2) as smp,
            tc.tile_pool(name="ot", bufs=3) as otp,
            tc.tile_pool(name="ys", bufs=2) as ysp,
            tc.tile_pool(name="yrhs", bufs=30) as yrhsp,
            tc.tile_pool(name="psQ", bufs=2, space="PSUM") as psQ,
            tc.tile_pool(name="psQb", bufs=1, space="PSUM") as psQb,
            tc.tile_pool(name="psS", bufs=3, space="PSUM") as psS,
            tc.tile_pool(name="psB", bufs=2, space="PSUM") as psB,
        ):
            # ---- warmup collective first: pure rendezvous ----
            nc.gpsimd.collective_compute(
                "AllGather",
                mybir.AluOpType.bypass,
                replica_groups=[list(range(NCORES))],
                ins=[agin_w.ap().opt()],
                outs=[agout_w.ap().opt()],
            )

            # rope tables, feature-major: [128 hd-part, S]
            cos_sb = constp.tile([128, S], bf16, tag="cos")
            sin_sb = constp.tile([128, S], bf16, tag="sin")
            nc.sync.dma_start(cos_sb[:], cosf[:, :])
            nc.sync.dma_start(sin_sb[:], sinf[:, :])

            ones = constp.tile([128, 128], bf16, tag="ones")
            nc.vector.memset(ones[:], 1.0)
            # 64-partition rotation matrix (RoPE pair swap in [r|i] layout)
            rotm = constp.tile([128, 128], bf16, tag="rotm")
            nc.sync.dma_start(rotm[:], rotmd[:, :])

            # resident weights: wcatT tiles [128d, 768], paired with the
            # first chunk's xT tiles so chunk-0 compute starts immediately
            xt_tiles = {}
            xt0 = xtp.tile([128, ND * 512], bf16, tag="xt", name="xt0")
            xt_tiles[0] = xt0
            # wcat packed [128, ND*768]: partition p holds all 16 d-tiles'
            # rows; loaded in 4-dt groups (6KB lines) interleaved with the
            # first chunk's x groups (8KB lines) so chunk-0 compute starts
            # while both stream in
            wcat_t = wpool.tile([128, ND * 3 * ESH], bf16, tag="wc",
                                name="wcat_t")
            wcat_sb = [wcat_t[:, dt * 768:(dt + 1) * 768] for dt in range(ND)]
            for g in range(4):
                nc.sync.dma_start(
                    wcat_t[:, g * 3072:(g + 1) * 3072],
                    wcatT[0:128, g * 3072:(g + 1) * 3072])
                nc.scalar.dma_start(
                    xt0[:, g * 2048:(g + 1) * 2048],
                    xTd[0:128, g * 2048:(g + 1) * 2048])

            # chunks 0-2 fill in 4-dt groups (compute starts while tiles
            # stream in); later chunks load as one dma (16KB lines)
            def fetch_chunk(tc_i):
                xt = xtp.tile([128, ND * 512], bf16, tag="xt", name="xtc")
                xt_tiles[tc_i] = xt
                r0 = tc_i * 128
                if tc_i < 3:
                    for g in range(4):
                        eng = nc.sync if g % 2 == 0 else nc.scalar
                        eng.dma_start(
                            xt[:, g * 2048:(g + 1) * 2048],
                            xTd[r0:r0 + 128, g * 2048:(g + 1) * 2048])
                else:
                    nc.scalar.dma_start(xt[:], xTd[r0:r0 + 128, :])

            fetch_chunk(1)

            # transposed diagonal mask block [j-part, i-free]; every
            # 128x128 diagonal block of the causal mask is identical
            maskd_sb = constp.tile([128, 128], bf16, tag="maskd")
            nc.scalar.dma_start(maskd_sb[:], maskdT[:, :])

            # wo tiles: loaded lazily (first needed by proj at chunk 4)
            wo_sb = []

            def load_wo():
                for dt in range(ND):
                    t_ = wpool.tile([128, ESH], bf16, tag=f"wo{dt}",
                                    name=f"wo{dt}")
                    nc.scalar.dma_start(t_[:], woT[dt * 128:(dt + 1) * 128, :])
                    wo_sb.append(t_)

            # persistent attention operands (bf16)
            # qT/kT: [(h,b)] -> [128hd, S]  (head-major, feature on partitions)
            # v    : [(h,b)] -> [128s-in-tile, S/128 * HD] natural blocks
            qT = {}
            kT = {}
            vN = {}
            for h in range(HPC):
                for b in range(B):
                    qT[h, b] = qkvp.tile([128, S], bf16, tag=f"q{h}{b}",
                                         name=f"qT{h}{b}")
                    kT[h, b] = qkvp.tile([128, S], bf16, tag=f"k{h}{b}",
                                         name=f"kT{h}{b}")
                    vN[h, b] = qkvp.tile([128, S // 128 * HD], bf16,
                                         tag=f"v{h}{b}", name=f"vN{h}{b}")

            # ---------------- QKV + RoPE ----------------
            # feature-major rope: psum q tile [128 hd, 512 tok] for one head;
            # hd permuted host-side to [64 reals | 64 imags].
            #   qc = psum * cos ; qs = psum * sin2  (sin2 = [+sin | -sin])
            #   dst = qc + rotate64(qs)
            # the 64-partition rotation runs on the PE (tiny matmul with a
            # permutation stationary) since DVE ops can't cross partitions;
            # the rot matmul is emitted a group later so its DVE deps are
            # ready by the time the PE reaches it.
            def rope_muls(ps_ap, span):
                qc = ropep.tile([128, 512], bf16, tag="qc")
                qs = ropep.tile([128, 512], bf16, tag="qs")
                nc.vector.tensor_mul(qc[:], ps_ap, cos_sb[:, span])
                nc.vector.tensor_mul(qs[:], ps_ap, sin_sb[:, span])
                return qc, qs

            def rope_rot(qc, qs, dst, span):
                prot = psQ.tile([128, 512], f32, tag="pa", name="prot")
                nc.tensor.matmul(prot[:], rotm[:], qs[:],
                                 start=True, stop=True)
                nc.vector.tensor_add(dst[:, span], qc[:], prot[:])

            def drain_v_pair(pv_ap, b, st0):
                # pv_ap holds [128 tok, 2tt * 2h * HD/... ] = [:,0:256]=tt0
                for half, st in ((0, st0), (1, st0 + 1)):
                    for h in range(HPC):
                        nc.scalar.activation(
                            vN[h, b][:, st * HD:(st + 1) * HD],
                            pv_ap[:, half * 256 + h * HD:
                                  half * 256 + (h + 1) * HD],
                            mybir.ActivationFunctionType.Copy)

            # dt-outer QKV: all six accumulation groups advance together
            # per dt (borrows psS banks) so compute starts as tiles arrive.
            # NOTE: start=True clears has_written for the WHOLE psum bank;
            # when two sub-groups share a bank only the first may set start.
            def qkv_chunk_dtouter(tc_i):
                b = tc_i // (S // 512)
                st0 = (tc_i * 4) % (S // 128)
                span = slice((tc_i % 4) * 512, (tc_i % 4) * 512 + 512)
                pq = [psQ.tile([128, 512], f32, tag="pa", name=f"pq{i}")
                      for i in range(2)]
                pk = [psS.tile([128, 512], f32, tag="S", name=f"pk{i}")
                      for i in range(2)]
                pva = psQb.tile([128, 512], f32, tag="pb")
                pvb = psS.tile([128, 512], f32, tag="S")
                for dt in range(ND):
                    xt = xt_tiles[tc_i][:, dt * 512:(dt + 1) * 512]
                    st_, sp_ = (dt == 0), (dt == ND - 1)
                    for f in range(2):
                        nc.tensor.matmul(
                            pq[f][:], wcat_sb[dt][:, f * 128:(f + 1) * 128],
                            xt[:], start=st_, stop=sp_)
                    for f in range(2):
                        nc.tensor.matmul(
                            pk[f][:],
                            wcat_sb[dt][:, 256 + f * 128:256 + (f + 1) * 128],
                            xt[:], start=st_, stop=sp_)
                    for tt in range(4):
                        dst = pva if tt < 2 else pvb
                        nc.tensor.matmul(
                            dst[:, (tt % 2) * 256:(tt % 2) * 256 + 256],
                            xt[:, tt * 128:(tt + 1) * 128],
                            wcat_sb[dt][:, 512:768],
                            start=(st_ and tt % 2 == 0), stop=sp_,
                            skip_group_check=True)
                mres = []
                for h in range(HPC):
                    mres.append((rope_muls(pq[h][:], span), qT[h, b]))
                for h in range(HPC):
                    mres.append((rope_muls(pk[h][:], span), kT[h, b]))
                for (qc, qs), dst in mres:
                    rope_rot(qc, qs, dst, span)
                drain_v_pair(pva[:], b, st0)
                drain_v_pair(pvb[:], b, st0 + 2)

            # ---------------- attention phases ----------------
            # one scores j-tile: matmul + (diag) mask add + exp + denominator
            # accumulation on DVE
            def score_tile(h, b, c, jt, acc, pts):
                lo = max(0, jt * 128 - c * 512)
                ps = psS.tile([128, 512], f32, tag="S")
                nc.tensor.matmul(
                    ps[:, lo:512],
                    kT[h, b][:, jt * 128:(jt + 1) * 128],
                    qT[h, b][:, c * 512 + lo:(c + 1) * 512],
                    start=True, stop=True)
                if jt >= 4 * c:
                    # diagonal block: add transposed mask
                    dl = jt * 128 - c * 512
                    nc.vector.tensor_add(
                        ps[:, dl:dl + 128], ps[:, dl:dl + 128],
                        maskd_sb[:])
                pt = ptp.tile([128, 512], bf16, tag="pt")
                nc.scalar.activation(
                    pt[:, lo:512], ps[:, lo:512],
                    mybir.ActivationFunctionType.Exp, scale=SCALE)
                if jt == 0:
                    nc.vector.tensor_scalar_mul(
                        acc[:, lo:512], pt[:, lo:512], 1.0)
                else:
                    nc.vector.tensor_add(
                        acc[:, lo:512], acc[:, lo:512], pt[:, lo:512])
                pts.append((pt, lo))

            # PV accumulation matmuls for one head (interleavable thunks)
            def pv_mms(h, b, c, pts, po):
                J = 4 * (c + 1)
                out = []
                for jt in range(J):
                    pt, lo = pts[jt]
                    out.append(lambda jt=jt, pt=pt, lo=lo: nc.tensor.matmul(
                        po[:, lo:512], vN[h, b][:, jt * HD:(jt + 1) * HD],
                        pt[:, lo:512], start=(jt == 0), stop=(jt == J - 1),
                        skip_group_check=True))
                return out

            # denominator matmul (cross-partition sum of acc, replicated)
            def denom_mm(acc):
                accb = accbp.tile([128, 512], bf16, tag="accb")
                nc.vector.tensor_scalar_mul(accb[:], acc[:], 1.0)
                pr = psB.tile([128, 512], f32, tag="B")
                nc.tensor.matmul(pr[:], ones[:], accb[:],
                                 start=True, stop=True)
                return pr

            # reciprocal -> normalize -> bounce write (per head half)
            def finish_phase(ci, h, pr, po):
                rbc = smp.tile([128, 512], f32, tag="rbc")
                nc.vector.reciprocal_approx_fast(rbc[:], pr[:])
                ot = otp.tile([128, 512], bf16, tag="ot")
                nc.vector.tensor_mul(ot[:], po[:], rbc[:])
                nc.scalar.dma_start(agin[ci][h * 128:(h + 1) * 128, :], ot[:])

            def all_gather(ci):
                nc.gpsimd.collective_compute(
                    "AllGather",
                    mybir.AluOpType.bypass,
                    replica_groups=[list(range(NCORES))],
                    ins=[agin[ci].ap().opt()],
                    outs=[agout[ci].ap().opt()],
                )

            # ---------------- output projection ----------------
            # NOTE: these loads depend on an AllGather; they must go on the
            # sync queue ONLY. A dma_start waiting on a collective would
            # head-of-line-block every later instruction on its engine
            # queue -- on the scalar queue that stalls all exp drains and
            # with them the whole pipeline when the fabric is slow.
            def rhs_load(k):
                rhs_tiles = []
                for et in range(ND):
                    rt = yrhsp.tile([128, 512], bf16, tag="yr")
                    nc.sync.dma_start(
                        rt[:], agout[k][et * 128:(et + 1) * 128, :])
                    rhs_tiles.append(rt)
                return rhs_tiles

            # projection emitted as thunks so its matmuls can fill PE gaps
            # between exp-gated score/PV tiles; psum borrowed from the v
            # bank (psQb), which is idle outside the QKV phase
            def project_thunks(k, rhs_tiles):
                thunks = []
                for ft in range(ESH // 128):
                    box = {}
                    for et in range(ND):
                        def mm(ft=ft, et=et, box=box):
                            if et == 0:
                                box["py"] = psQb.tile([128, 512], f32,
                                                      tag="pb", name="py")
                            nc.tensor.matmul(
                                box["py"][:],
                                wo_sb[et][:, ft * 128:(ft + 1) * 128],
                                rhs_tiles[et][:],
                                start=(et == 0), stop=(et == ND - 1))
                        thunks.append(mm)

                    def drain(ft=ft, box=box, k=k):
                        ys = ysp.tile([128, 512], f32, tag="ys")
                        nc.scalar.activation(
                            ys[:], box["py"][:],
                            mybir.ActivationFunctionType.Copy)
                        nc.scalar.dma_start(
                            yT[ft * 128:(ft + 1) * 128,
                               k * 512:(k + 1) * 512],
                            ys[:])
                    thunks.append(drain)
                return thunks

            # ---------------- fused pipeline ----------------
            rhs_pre = {}
            for ci in range(NCHUNK):
                b, c = divmod(ci, S // 512)
                J = 4 * (c + 1)
                if 1 <= ci and ci + 1 < NCHUNK:
                    fetch_chunk(ci + 1)
                if ci == 3:
                    load_wo()
                if ci == 4:
                    rhs_pre[0] = rhs_load(0)
                filler = []
                for k in {5: [0], 6: [1, 2], 7: [3]}.get(ci, []):
                    filler += project_thunks(
                        k, rhs_pre.pop(k) if k in rhs_pre else rhs_load(k))

                def pop_filler(n=1):
                    for _ in range(min(n, len(filler))):
                        filler.pop(0)()

                qkv_chunk_dtouter(ci)
                # head0 scores, with projection filler between tiles
                last = ci == NCHUNK - 1
                pts0 = []
                acc0 = accp.tile([128, 512], f32, tag="acc")
                for jt in range(J):
                    score_tile(0, b, c, jt, acc0, pts0)
                    if not last:
                        pop_filler()
                pr0 = denom_mm(acc0)
                # head1 scores interleaved with head0 PV
                po0 = psB.tile([128, 512], f32, tag="B")
                pv0 = pv_mms(0, b, c, pts0, po0)
                pts1 = []
                acc1 = accp.tile([128, 512], f32, tag="acc")
                for jt in range(J):
                    score_tile(1, b, c, jt, acc1, pts1)
                    pv0[jt]()
                finish_phase(ci, 0, pr0, po0)
                pr1 = denom_mm(acc1)
                # head1 PV interleaved with projection filler
                po1 = psB.tile([128, 512], f32, tag="B")
                pv1 = pv_mms(1, b, c, pts1, po1)
                for jt in range(J):
                    pv1[jt]()
                    if not last:
                        pop_filler()
                finish_phase(ci, 1, pr1, po1)
                all_gather(ci)
                pop_filler(len(filler))
                if last:
                    for k in (NCHUNK - 4, NCHUNK - 3):
                        rhs_pre[k] = rhs_load(k)
            for k in range(NCHUNK - 4, NCHUNK):
                rhs = rhs_pre.pop(k) if k in rhs_pre else rhs_load(k)
                for th in project_thunks(k, rhs):
                    th()
    nc.finalize()
    return nc


def _prep_inputs(x, wq, wk, wv, wo, freqs_cos, freqs_sin, mask):
    bf16 = ml_dtypes.bfloat16
    xf = np.ascontiguousarray(x.reshape(T, D).T).astype(bf16)
    # pre-tiled [NCHUNK, 128, ND, 512]: each SBUF partition's whole-chunk
    # data (16 dt-slices) is one contiguous 16KB DRAM row, so a chunk
    # loads as ONE dma with 16KB lines instead of 16 dmas of 1KB lines
    xf = np.ascontiguousarray(
        xf.reshape(ND, 128, NCHUNK, 512).transpose(2, 1, 0, 3)
    ).reshape(NCHUNK * 128, ND * 512)
    # rope tables, feature-major [128, S]:
    #   rows 0..63 = pair j (reals), rows 64..127 = pair j (imags)
    #   cos rows duplicate cos[:, j]; sin rows carry the rotation signs:
    #   sin2[0:64] = +sin (applied to swapped imag->real term read from
    #   the imag block), sin2[64:128] = -sin.
    cosf = np.concatenate([freqs_cos.T, freqs_cos.T], axis=0).astype(bf16)
    sinf = np.concatenate([freqs_sin.T, -freqs_sin.T], axis=0).astype(bf16)
    # 64-partition rotation: out[p] = in[(p+64)%128] as out = rotm.T @ in
    rotmat = np.zeros((128, 128), dtype=bf16)
    rotmat[(np.arange(128) + 64) % 128, np.arange(128)] = 1
    # one transposed diagonal mask block (all diagonal blocks identical)
    mdT = np.ascontiguousarray(mask[0:128, 0:128].T).astype(bf16)
    # head-dim permutation for q/k: [evens | odds] per head
    hdperm = np.concatenate([np.arange(0, HD, 2), np.arange(1, HD, 2)])
    in_maps = []
    for cidx in range(NCORES):
        sl = slice(cidx * ESH, (cidx + 1) * ESH)
        wqc, wkc = wq[sl, :], wk[sl, :]
        qperm = np.concatenate([h * HD + hdperm for h in range(HPC)])
        wcatT = np.concatenate(
            [wqc[qperm, :].T, wkc[qperm, :].T, wv[sl, :].T],
            axis=1).astype(bf16)
        # pack [D, 768] -> [128, ND*768]: partition p holds its 16 tile-rows
        wcatT = np.ascontiguousarray(
            wcatT.reshape(ND, 128, 768).transpose(1, 0, 2)
        ).reshape(128, ND * 768)
        woTc = np.ascontiguousarray(wo[sl, :].T).astype(bf16)
        in_maps.append({
            "xT": xf,
            "wcatT": np.ascontiguousarray(wcatT),
            "woT": woTc,
            "cosf": cosf,
            "sinf": sinf,
            "rotm": rotmat,
            "maskdT": mdT,
        })
    return in_maps


def kernel(x, wq, wk, wv, wo, freqs_cos, freqs_sin, mask, start_pos):
    from concourse.bass_utils import run_bass_kernel_spmd

    x = np.asarray(x, dtype=np.float32)
    in_maps = _prep_inputs(
        np.asarray(x, np.float32), np.asarray(wq, np.float32),
        np.asarray(wk, np.float32), np.asarray(wv, np.float32),
        np.asarray(wo, np.float32), np.asarray(freqs_cos, np.float32),
        np.asarray(freqs_sin, np.float32), np.asarray(mask, np.float32))

    if "nc" not in _cache:
        _cache["nc"] = _build_nc()
    res = run_bass_kernel_spmd(_cache["nc"], in_maps, core_ids=list(range(NCORES)))
    _cache["last_result"] = res

    y = np.empty((T, D), dtype=np.float32)
    for c in range(NCORES):
        y[:, c * ESH:(c + 1) * ESH] = np.asarray(res.results[c]["out"]).T
    return y.reshape(B, S, D)
